# revision 5
# baseline (speedup 1.0000x reference)
"""TRN2 Bass kernel for nn_Block_6476810682806 (dense_cnn).

Bottleneck block: 1x1 kerv -> BN -> 3x3 kerv -> BN -> 1x1 kerv -> BN -> +residual,
where kerv(x) = (conv(x) + 1)^2 and BN is inference-mode (frozen stats).

Distribution: data-parallel over batch (128 -> 16 per core) across 8 cores,
weights replicated. Each core computes its shard fully independently.

Device strategy (per core):
  - activations channel-major: [C partitions, batch*spatial free]
  - convs as PE matmuls in float32r (fp32 data, 1 cyc/row at N>=256)
  - 3x3 conv: 9 shifted matmuls over a zero-padded per-image 16x16 SBUF plane
  - BN scale folded into the kervolution square on ACT:
        s*(y+1)^2 = (sqrt(s)*y + sqrt(s))^2  (requires s > 0)
    shifts (t = b - m*s) are zero for this problem's fills; generic paths
    emit an extra per-channel add / affine when they are not.
  - residual add on DVE, straight from the resident x tiles
"""

import numpy as np

import concourse.bacc as bacc
import concourse.mybir as mybir
import concourse.tile as tile

F32 = mybir.dt.float32
F32R = mybir.dt.float32r
EPS = 1e-5

B = 16          # images per core
C_IN = 1024
C_MID = 256
HW = 14
S = HW * HW     # 196
PASSES = 4
BP = B // PASSES          # images per pass = 4
NT = BP // 2              # n-tiles per pass (2 images each) = 2
N = 2 * S                 # matmul moving size = 392
PAD = 16                  # padded plane side
PS = PAD * PAD            # 256 padded plane size
K1 = C_IN // 128          # 8
K2 = C_MID // 128         # 2
M1 = C_MID // 128         # 2
M3 = C_IN // 128          # 8

# layer modes
FAST_T0 = 0   # all s>0, all t==0: ACT-only pointwise
FAST_T = 1    # all s>0, some t!=0: ACT + per-channel add
SLOW = 2      # some s<=0: plain square on ACT + DVE affine


def _build(modes):
    mode1, mode2, mode3 = modes
    nc = bacc.Bacc("TRN2", target_bir_lowering=False, debug=False)

    x_d = nc.dram_tensor("x", [B, C_IN, HW, HW], F32, kind="ExternalInput").ap()
    w1_d = nc.dram_tensor("w1t", [C_IN, C_MID], F32, kind="ExternalInput").ap()
    w2_d = nc.dram_tensor("w2t", [K2, 9, 128, C_MID], F32, kind="ExternalInput").ap()
    w3_d = nc.dram_tensor("w3t", [C_MID, C_IN], F32, kind="ExternalInput").ap()
    # per-m-tile [128, m] act scale / act bias / extra shift per layer
    sc1_d = nc.dram_tensor("sc1", [128, M1], F32, kind="ExternalInput").ap()
    bi1_d = nc.dram_tensor("bi1", [128, M1], F32, kind="ExternalInput").ap()
    sh1_d = nc.dram_tensor("sh1", [128, M1], F32, kind="ExternalInput").ap()
    sc2_d = nc.dram_tensor("sc2", [128, M1], F32, kind="ExternalInput").ap()
    bi2_d = nc.dram_tensor("bi2", [128, M1], F32, kind="ExternalInput").ap()
    sh2_d = nc.dram_tensor("sh2", [128, M1], F32, kind="ExternalInput").ap()
    sc3_d = nc.dram_tensor("sc3", [128, M3], F32, kind="ExternalInput").ap()
    bi3_d = nc.dram_tensor("bi3", [128, M3], F32, kind="ExternalInput").ap()
    sh3_d = nc.dram_tensor("sh3", [128, M3], F32, kind="ExternalInput").ap()
    out_d = nc.dram_tensor("out", [B, C_IN, HW, HW], F32, kind="ExternalOutput").ap()

    x_cm = x_d.rearrange("n c h w -> c n (h w)")      # [1024, 16, 196]
    out_cm = out_d.rearrange("n c h w -> c n (h w)")  # [1024, 16, 196]

    Sq = mybir.ActivationFunctionType.Square
    Alu = mybir.AluOpType

    with tile.TileContext(nc) as tc:
        with (
            tc.tile_pool(name="wpool", bufs=1) as wpool,
            tc.tile_pool(name="xpool", bufs=2) as xpool,
            tc.tile_pool(name="h1pool", bufs=2) as h1pool,
            tc.tile_pool(name="h2pool", bufs=2) as h2pool,
            tc.tile_pool(name="opool", bufs=4) as opool,
            tc.tile_pool(name="ps1", bufs=2, space="PSUM") as ps1pool,
            tc.tile_pool(name="ps2", bufs=2, space="PSUM") as ps2pool,
            tc.tile_pool(name="ps3", bufs=4, space="PSUM") as ps3pool,
        ):
            # ---- weights + scales (resident) ----
            w1t = []
            for k in range(K1):
                t = wpool.tile([128, C_MID], F32R, tag=f"w1_{k}")
                nc.sync.dma_start(t[:], w1_d[k * 128:(k + 1) * 128, :].bitcast(F32R))
                w1t.append(t)
            w2t = []
            for k in range(K2):
                for tp in range(9):
                    t = wpool.tile([128, C_MID], F32R, tag=f"w2_{k}_{tp}")
                    nc.sync.dma_start(t[:], w2_d[k, tp].bitcast(F32R))
                    w2t.append(t)
            w3t = []
            for k in range(K2):
                t = wpool.tile([128, C_IN], F32R, tag=f"w3_{k}")
                nc.sync.dma_start(t[:], w3_d[k * 128:(k + 1) * 128, :].bitcast(F32R))
                w3t.append(t)

            def load_vec(ap_d, m, name):
                t = wpool.tile([128, m], F32, tag=name)
                nc.sync.dma_start(t[:], ap_d)
                return t

            sc1 = load_vec(sc1_d, M1, "sc1")
            bi1 = load_vec(bi1_d, M1, "bi1")
            sc2 = load_vec(sc2_d, M1, "sc2")
            bi2 = load_vec(bi2_d, M1, "bi2")
            sc3 = load_vec(sc3_d, M3, "sc3")
            bi3 = load_vec(bi3_d, M3, "bi3")
            sh1 = load_vec(sh1_d, M1, "sh1") if mode1 != FAST_T0 else None
            sh2 = load_vec(sh2_d, M1, "sh2") if mode2 != FAST_T0 else None
            sh3 = load_vec(sh3_d, M3, "sh3") if mode3 != FAST_T0 else None

            def pointwise(mode, ps, out_ap, sct, bit, sht, m):
                """out = s*(ps+1)^2 + t, written to out_ap (shape-matching AP)."""
                if mode == SLOW:
                    # plain (y+1)^2 then affine on DVE
                    nc.scalar.activation(out_ap, ps[:], Sq, bias=1.0, scale=1.0)
                    nc.vector.tensor_scalar(
                        out_ap, out_ap, sct[:, m:m + 1], sht[:, m:m + 1],
                        Alu.mult, Alu.add)
                else:
                    nc.scalar.activation(out_ap, ps[:], Sq,
                                         bias=bit[:, m:m + 1], scale=sct[:, m:m + 1])
                    if mode == FAST_T:
                        nc.vector.tensor_scalar(
                            out_ap, out_ap, sht[:, m:m + 1], None, Alu.add)

            # ---- main passes ----
            for p in range(PASSES):
                # x tiles for this pass: [128, BP*S] channel-major
                xt = []
                for k in range(K1):
                    t = xpool.tile([128, BP * S], F32R, tag=f"x{k}")
                    src = x_cm[k * 128:(k + 1) * 128, p * BP:(p + 1) * BP, :]
                    nc.sync.dma_start(t[:].rearrange("c (n s) -> c n s", n=BP),
                                      src.bitcast(F32R))
                    xt.append(t)

                # padded h1 planes (zeroed borders)
                h1 = []
                for k in range(K2):
                    t = h1pool.tile([128, BP * PS], F32R, tag=f"h1_{k}")
                    nc.gpsimd.memset(t[:].bitcast(F32), 0.0)
                    h1.append(t)
                h2 = []
                for k in range(K2):
                    h2t = h2pool.tile([128, BP * S], F32R, tag=f"h2_{k}")
                    h2.append(h2t)

                # conv1: 1x1, C_IN -> C_MID
                for m in range(M1):
                    for j in range(NT):
                        ps = ps1pool.tile([128, N], F32, tag="ps1")
                        for k in range(K1):
                            nc.tensor.matmul(
                                ps[:],
                                w1t[k][:, m * 128:(m + 1) * 128],
                                xt[k][:, j * N:(j + 1) * N],
                                start=(k == 0), stop=(k == K1 - 1))
                        # write z1 into padded interior of images 2j, 2j+1
                        dst = (h1[m][:]
                               .rearrange("c (n a b) -> c n a b", a=PAD, b=PAD)
                               [:, 2 * j:2 * j + 2, 1:1 + HW, 1:1 + HW])
                        pointwise(mode1, ps, dst, sc1, bi1, sh1, m)

                # conv2: 3x3 pad 1, C_MID -> C_MID
                h1v = [t[:].rearrange("c (n a b) -> c n a b", a=PAD, b=PAD)
                       for t in h1]
                for m in range(M1):
                    for j in range(NT):
                        ps = ps2pool.tile([128, N], F32, tag="ps2")
                        first = True
                        for k in range(K2):
                            for tp in range(9):
                                kh, kw = tp // 3, tp % 3
                                rhs = h1v[k][:, 2 * j:2 * j + 2,
                                             kh:kh + HW, kw:kw + HW]
                                nc.tensor.matmul(
                                    ps[:],
                                    w2t[k * 9 + tp][:, m * 128:(m + 1) * 128],
                                    rhs,
                                    start=first, stop=(k == K2 - 1 and tp == 8))
                                first = False
                        dst = h2[m][:, j * N:(j + 1) * N]
                        pointwise(mode2, ps, dst, sc2, bi2, sh2, m)

                # conv3: 1x1, C_MID -> C_IN, + residual, store
                for m in range(M3):
                    for j in range(NT):
                        ps = ps3pool.tile([128, N], F32, tag="ps3")
                        for k in range(K2):
                            nc.tensor.matmul(
                                ps[:],
                                w3t[k][:, m * 128:(m + 1) * 128],
                                h2[k][:, j * N:(j + 1) * N],
                                start=(k == 0), stop=(k == K2 - 1))
                        zt = opool.tile([128, N], F32, tag="z")
                        pointwise(mode3, ps, zt[:], sc3, bi3, sh3, m)
                        nc.vector.tensor_tensor(
                            zt[:], zt[:], xt[m][:, j * N:(j + 1) * N].bitcast(F32), Alu.add)
                        dst = out_cm[m * 128:(m + 1) * 128,
                                     p * BP + 2 * j:p * BP + 2 * j + 2, :]
                        nc.sync.dma_start(
                            dst, zt[:].rearrange("c (n s) -> c n s", n=2))

    nc.compile()
    return nc


# ---------------- host side ----------------

_CACHE = {}


def _get_runner(modes):
    if modes in _CACHE:
        return _CACHE[modes]
    import jax
    from jax.experimental.shard_map import shard_map
    from jax.sharding import Mesh, PartitionSpec
    from concourse.bass2jax import (_bass_exec_p, install_neuronx_cc_hook,
                                    partition_id_tensor)

    nc = _build(modes)
    install_neuronx_cc_hook()
    partition_name = nc.partition_id_tensor.name if nc.partition_id_tensor else None
    in_names, out_names, out_avals = [], [], []
    for alloc in nc.m.functions[0].allocations:
        if not isinstance(alloc, mybir.MemoryLocationSet):
            continue
        name = alloc.memorylocations[0].name
        if alloc.kind == "ExternalInput":
            if name != partition_name:
                in_names.append(name)
        elif alloc.kind == "ExternalOutput":
            out_names.append(name)
            out_avals.append(jax.core.ShapedArray(
                tuple(alloc.tensor_shape), mybir.dt.np(alloc.dtype)))
    n_params, n_outs = len(in_names), len(out_avals)
    all_in_names = list(in_names) + list(out_names)
    if partition_name is not None:
        all_in_names.append(partition_name)

    def _body(*args):
        operands = list(args)
        if partition_name is not None:
            operands.append(partition_id_tensor())
        outs = _bass_exec_p.bind(
            *operands,
            out_avals=tuple(out_avals),
            in_names=tuple(all_in_names),
            out_names=tuple(out_names),
            lowering_input_output_aliases=(),
            sim_require_finite=True,
            sim_require_nnan=True,
            nc=nc,
        )
        return tuple(outs)

    devices = jax.devices()[:8]
    mesh = Mesh(np.asarray(devices), ("core",))
    sharded = jax.jit(
        shard_map(_body, mesh=mesh,
                  in_specs=(PartitionSpec("core"),) * (n_params + n_outs),
                  out_specs=(PartitionSpec("core"),) * n_outs,
                  check_rep=False),
        donate_argnums=tuple(range(n_params, n_params + n_outs)),
        keep_unused=True,
    )
    sharding = jax.sharding.NamedSharding(mesh, PartitionSpec("core"))
    runner = dict(nc=nc, sharded=sharded, sharding=sharding, jax=jax,
                  in_names=in_names, out_names=out_names, out_avals=out_avals)
    _CACHE[modes] = runner
    return runner


def _vec_tile(v, m_tiles):
    """[C] -> [128, m_tiles] column-per-m-tile layout."""
    return np.ascontiguousarray(v.reshape(m_tiles, 128).T.astype(np.float32))


def prepare(w1, w2, w3, g1, b1, m1, v1, g2, b2, m2, v2, g3, b3, m3, v3):
    """Host prep: returns (modes, shared_input_dict_without_x, x_full)."""
    s1 = g1 / np.sqrt(v1 + EPS)
    t1 = b1 - m1 * s1
    s2 = g2 / np.sqrt(v2 + EPS)
    t2 = b2 - m2 * s2
    s3 = g3 / np.sqrt(v3 + EPS)
    t3 = b3 - m3 * s3

    def mode_of(s, t):
        if np.all(s > 0):
            return FAST_T0 if not np.any(t) else FAST_T
        return SLOW

    modes = (mode_of(s1, t1), mode_of(s2, t2), mode_of(s3, t3))

    def sc_bi(mode, s, t, m_tiles):
        if mode == SLOW:
            sc = _vec_tile(s, m_tiles)          # DVE affine scale
            bi = np.ones((128, m_tiles), np.float32)
            sh = _vec_tile(t, m_tiles)
        else:
            r = np.sqrt(s)
            sc = _vec_tile(r, m_tiles)
            bi = _vec_tile(r, m_tiles)
            sh = _vec_tile(t, m_tiles)
        return sc, bi, sh

    sc1, bi1, sh1 = sc_bi(modes[0], s1, t1, M1)
    sc2, bi2, sh2 = sc_bi(modes[1], s2, t2, M1)
    sc3, bi3, sh3 = sc_bi(modes[2], s3, t3, M3)

    w1t = np.ascontiguousarray(w1[:, :, 0, 0].T.astype(np.float32))      # [1024,256]
    # w2: [o, i, kh, kw] -> [k, tap, i_local, o]
    w2t = np.ascontiguousarray(
        w2.transpose(1, 2, 3, 0)                  # [i, kh, kw, o]
          .reshape(K2, 128, 9, C_MID)             # [k, i_local, tap, o]
          .transpose(0, 2, 1, 3)                  # [k, tap, i_local, o]
          .astype(np.float32))
    w3t = np.ascontiguousarray(w3[:, :, 0, 0].T.astype(np.float32))      # [256,1024]

    shared = dict(w1t=w1t, w2t=w2t, w3t=w3t,
                  sc1=sc1, bi1=bi1, sh1=sh1,
                  sc2=sc2, bi2=bi2, sh2=sh2,
                  sc3=sc3, bi3=bi3, sh3=sh3)
    return modes, shared


def kernel(**inputs):
    inputs = {k: np.asarray(v) for k, v in inputs.items()}
    x = inputs.pop("x").astype(np.float32)
    modes, shared = prepare(**inputs)
    r = _get_runner(modes)
    jax = r["jax"]

    n_cores = 8
    # concat per-core inputs on axis 0 (x sharded, everything else replicated)
    dev_in = []
    for name in r["in_names"]:
        if name == "x":
            cat = x  # [128, ...] == 8 cores x 16
        else:
            a = shared[name]
            cat = np.concatenate([a] * n_cores, axis=0)
        dev_in.append(jax.device_put(cat, r["sharding"]))
    zero_outs = [
        jax.device_put(np.zeros((n_cores * av.shape[0], *av.shape[1:]), av.dtype),
                       r["sharding"])
        for av in r["out_avals"]
    ]
    outs = r["sharded"](*dev_in, *zero_outs)
    jax.block_until_ready(outs)
    out = np.asarray(outs[r["out_names"].index("out")])
    return out.reshape(128, C_IN, HW, HW)


# revision 24
# speedup vs baseline: 1.2185x; 1.2185x over previous
"""TRN2 Bass kernel for nn_Block_6476810682806 (dense_cnn).

Bottleneck block: 1x1 kerv -> BN -> 3x3 kerv -> BN -> 1x1 kerv -> BN -> +residual,
where kerv(x) = (conv(x) + 1)^2 and BN is inference-mode (frozen stats).

Distribution: data-parallel over batch (128 -> 16 per core) across 8 cores,
weights replicated. Each core computes its shard fully independently.

Device strategy (per core):
  - activations channel-major: [C partitions, batch*spatial free]
  - convs as PE matmuls in float32r (fp32 data, 1 cyc/row at N>=256)
  - 3x3 conv: 9 shifted matmuls over a zero-padded per-image 16x16 SBUF plane
  - BN scale folded into the kervolution square on ACT:
        s*(y+1)^2 = (sqrt(s)*y + sqrt(s))^2  (requires s > 0)
    shifts (t = b - m*s) are zero for this problem's fills; generic paths
    emit an extra per-channel add / affine when they are not.
  - residual add on DVE, straight from the resident x supertiles
  - DMAs batched into supertile transfers (HWDGE fixed cost per DMA is ~0.6us)
"""

import numpy as np

import concourse.bacc as bacc
import concourse.mybir as mybir
import concourse.tile as tile

F32 = mybir.dt.float32
F32R = mybir.dt.float32r
EPS = 1e-5

B = 16          # images per core
C_IN = 1024
C_MID = 256
HW = 14
S = HW * HW     # 196
PASSES = 4
BP = B // PASSES          # images per pass = 4
NT = BP // 2              # n-tiles per pass (2 images each) = 2
N = 2 * S                 # matmul moving size = 392
PAD = 16                  # padded plane side
PS = PAD * PAD            # 256 padded plane size
K1 = C_IN // 128          # 8
K2 = C_MID // 128         # 2
M1 = C_MID // 128         # 2
M3 = C_IN // 128          # 8

# layer modes
FAST_T0 = 0   # all s>0, all t==0: ACT-only pointwise
FAST_T = 1    # all s>0, some t!=0: ACT + per-channel add
SLOW = 2      # some s<=0: plain square on ACT + DVE affine

# packed scale/bias column offsets in scb [128, 24]
SC1, BI1, SC2, BI2, SC3, BI3 = 0, 2, 4, 6, 8, 16
# packed shift column offsets in shb [128, 12]
SH1, SH2, SH3 = 0, 2, 4


def _build(modes, reps=None):
    mode1, mode2, mode3 = modes
    nc = bacc.Bacc("TRN2", target_bir_lowering=False, debug=False)

    x_d = nc.dram_tensor("x", [B, C_IN, HW, HW], F32, kind="ExternalInput").ap()
    w1_d = nc.dram_tensor("w1t", [C_IN, C_MID], F32, kind="ExternalInput").ap()
    w2_d = nc.dram_tensor("w2t", [K2, 9, 128, C_MID], F32, kind="ExternalInput").ap()
    w3_d = nc.dram_tensor("w3t", [C_MID, C_IN], F32, kind="ExternalInput").ap()
    scb_d = nc.dram_tensor("scb", [128, 24], F32, kind="ExternalInput").ap()
    shb_d = nc.dram_tensor("shb", [128, 12], F32, kind="ExternalInput").ap()
    out_d = nc.dram_tensor("out", [B, C_IN, HW, HW], F32, kind="ExternalOutput").ap()

    # x in (image, k-tile) column order: global column q = n*K1 + k, so the
    # DRAM strides merge into a single 3-dim DMA AP (k stride * K1 == n stride)
    x_nk = x_d.rearrange("n (k p) h w -> p (n k) (h w)", p=128)   # [128,128,196]
    out_cm = out_d.rearrange("n c h w -> c n (h w)")              # [1024,16,196]

    Sq = mybir.ActivationFunctionType.Square
    Alu = mybir.AluOpType

    with tile.TileContext(nc) as tc:
        with (
            tc.tile_pool(name="wpool", bufs=1) as wpool,
            tc.tile_pool(name="xpool", bufs=2) as xpool,
            tc.tile_pool(name="h1pool", bufs=2) as h1pool,
            tc.tile_pool(name="h2pool", bufs=2) as h2pool,
            tc.tile_pool(name="opool", bufs=4) as opool,
            tc.tile_pool(name="ps1", bufs=2, space="PSUM") as ps1pool,
            tc.tile_pool(name="ps2", bufs=2, space="PSUM") as ps2pool,
            tc.tile_pool(name="ps3", bufs=4, space="PSUM") as ps3pool,
        ):
            def xcol(xh, k, j):
                # [128, 2, S] rhs slice for k-tile k, image pair j
                v = xh[j][:].rearrange("p (n q) -> p n q", n=2)
                return v[:, :, k * S:(k + 1) * S]

            # ---- startup: interleave the serialized DMA stream in first-use
            # order: xj0, scale vec, w1, w2 first half, xj1, w2 rest, w3 ----
            def load_xj(pair, j):
                # pair: global image-pair index 0..7; j: slot parity in pass
                t = xpool.tile([128, 2 * K1 * S], F32R, tag=f"x{j}",
                               name=f"xt_q{pair}")
                c0 = 2 * pair * K1
                nc.sync.dma_start(
                    t[:].rearrange("p (q s) -> p q s", q=2 * K1),
                    x_nk[:, c0:c0 + 2 * K1, :].bitcast(F32R))
                return t

            # everything startup-critical on ONE queue (SP) in first-use
            # order, so later x prefetches cannot overtake weights on the
            # serialized DMA path
            xj0 = load_xj(0, 0)
            scb = wpool.tile([128, 24], F32, tag="scb")
            nc.sync.dma_start(scb[:], scb_d)
            if modes != (FAST_T0, FAST_T0, FAST_T0):
                shb = wpool.tile([128, 12], F32, tag="shb")
                nc.sync.dma_start(shb[:], shb_d)
            else:
                shb = None
            w1view = w1_d.rearrange("(k p) o -> p k o", p=128).bitcast(F32R)
            w1s = wpool.tile([128, K1 * C_MID], F32R, tag="w1s")
            w1v = w1s[:].rearrange("p (k o) -> p k o", k=K1)
            nc.sync.dma_start(w1v[:, 0:K1 // 2], w1view[:, 0:K1 // 2])
            nc.sync.dma_start(w1v[:, K1 // 2:], w1view[:, K1 // 2:])
            w2view = w2_d.rearrange("k t p o -> p (k t) o").bitcast(F32R)
            w2s = wpool.tile([128, 18 * C_MID], F32R, tag="w2s")
            w2v = w2s[:].rearrange("p (kt o) -> p kt o", kt=18)
            nc.sync.dma_start(w2v[:, 0:9], w2view[:, 0:9])
            nc.sync.dma_start(w2v[:, 9:18], w2view[:, 9:18])
            xj1 = load_xj(1, 1)
            xt0 = [xj0, xj1]
            w3s = wpool.tile([128, K2 * C_IN], F32R, tag="w3s")
            nc.sync.dma_start(
                w3s[:].rearrange("p (k o) -> p k o", k=K2),
                w3_d.rearrange("(k p) o -> p k o", p=128).bitcast(F32R))

            def w1ap(k, m):
                return w1s[:, k * C_MID + m * 128: k * C_MID + (m + 1) * 128]

            def w2ap(kt, m):
                return w2s[:, kt * C_MID + m * 128: kt * C_MID + (m + 1) * 128]

            def w3ap(k, m):
                return w3s[:, k * C_IN + m * 128: k * C_IN + (m + 1) * 128]

            def pointwise(mode, ps, out_ap, sc_off, sh_off, m):
                """out = s*(ps+1)^2 + t, written to out_ap (shape-matching AP)."""
                if mode == SLOW:
                    nc.scalar.activation(out_ap, ps[:], Sq, bias=1.0, scale=1.0)
                    nc.vector.tensor_scalar(
                        out_ap, out_ap, scb[:, sc_off + m:sc_off + m + 1],
                        shb[:, sh_off + m:sh_off + m + 1], Alu.mult, Alu.add)
                else:
                    nc.scalar.activation(
                        out_ap, ps[:], Sq,
                        bias=scb[:, sc_off + (M1 if sc_off < SC3 else M3) + m:
                                 sc_off + (M1 if sc_off < SC3 else M3) + m + 1],
                        scale=scb[:, sc_off + m:sc_off + m + 1])
                    if mode == FAST_T:
                        nc.vector.tensor_scalar(
                            out_ap, out_ap, shb[:, sh_off + m:sh_off + m + 1],
                            None, Alu.add)

            # ---- main passes: (first image pair index, n pairs) ----
            def emit_passes():
              plan = [(0, 2), (2, 2), (4, 2), (6, 2)]
              for pi, (q0, npairs) in enumerate(plan):
                xt = (xt0 if pi == 0 else
                      [load_xj(q0 + j, j) for j in range(npairs)])

                h1 = []
                for k in range(K2):
                    t = h1pool.tile([128, BP * PS], F32R, tag=f"h1_{k}")
                    nc.gpsimd.memset(t[:].bitcast(F32), 0.0)
                    h1.append(t)
                h2 = []
                for k in range(K2):
                    h2t = h2pool.tile([128, BP * S], F32R, tag=f"h2_{k}")
                    h2.append(h2t)

                # conv1: 1x1, C_IN -> C_MID (j outer: matches x arrival)
                for j in range(npairs):
                    for m in range(M1):
                        ps = ps1pool.tile([128, N], F32, tag="ps1")
                        for k in range(K1):
                            nc.tensor.matmul(
                                ps[:], w1ap(k, m), xcol(xt, k, j),
                                start=(k == 0), stop=(k == K1 - 1))
                        dst = (h1[m][:]
                               .rearrange("c (n a b) -> c n a b", a=PAD, b=PAD)
                               [:, 2 * j:2 * j + 2, 1:1 + HW, 1:1 + HW])
                        pointwise(mode1, ps, dst, SC1, SH1, m)

                # conv2: 3x3 pad 1, C_MID -> C_MID
                h1v = [t[:].rearrange("c (n a b) -> c n a b", a=PAD, b=PAD)
                       for t in h1]
                for j in range(npairs):
                    for m in range(M1):
                        ps = ps2pool.tile([128, N], F32, tag="ps2")
                        first = True
                        for k in range(K2):
                            for tp in range(9):
                                kh, kw = tp // 3, tp % 3
                                rhs = h1v[k][:, 2 * j:2 * j + 2,
                                             kh:kh + HW, kw:kw + HW]
                                nc.tensor.matmul(
                                    ps[:], w2ap(k * 9 + tp, m), rhs,
                                    start=first, stop=(k == K2 - 1 and tp == 8))
                                first = False
                        dst = h2[m][:, j * N:(j + 1) * N]
                        pointwise(mode2, ps, dst, SC2, SH2, m)

                # conv3: 1x1, C_MID -> C_IN, + residual, store per m
                for m in range(M3):
                    zt = opool.tile([128, npairs * N], F32, tag="z")
                    for j in range(npairs):
                        ps = ps3pool.tile([128, N], F32, tag="ps3")
                        for k in range(K2):
                            nc.tensor.matmul(
                                ps[:], w3ap(k, m), h2[k][:, j * N:(j + 1) * N],
                                start=(k == 0), stop=(k == K2 - 1))
                        zslice = zt[:, j * N:(j + 1) * N]
                        pointwise(mode3, ps, zslice, SC3, SH3, m)
                        zv = zslice.rearrange("c (n s) -> c n s", n=2)
                        nc.vector.tensor_tensor(
                            zv, zv, xcol(xt, m, j).bitcast(F32), Alu.add)
                    if pi == len(plan) - 1:
                        for j in range(npairs):
                            dst = out_cm[m * 128:(m + 1) * 128,
                                         2 * (q0 + j):2 * (q0 + j) + 2, :]
                            nc.sync.dma_start(
                                dst, zt[:, j * N:(j + 1) * N]
                                .rearrange("c (n s) -> c n s", n=2))
                    else:
                        dst = out_cm[m * 128:(m + 1) * 128,
                                     2 * q0:2 * q0 + 2 * npairs, :]
                        nc.sync.dma_start(
                            dst, zt[:].rearrange("c (n s) -> c n s", n=2 * npairs))

            if reps is None:
                emit_passes()
            else:
                with tc.For_i(0, reps, 1):
                    emit_passes()

    nc.compile()
    return nc


# ---------------- host side ----------------

_CACHE = {}


def _get_runner(modes):
    if modes in _CACHE:
        return _CACHE[modes]
    import jax
    from jax.experimental.shard_map import shard_map
    from jax.sharding import Mesh, PartitionSpec
    from concourse.bass2jax import (_bass_exec_p, install_neuronx_cc_hook,
                                    partition_id_tensor)

    nc = _build(modes)
    install_neuronx_cc_hook()
    partition_name = nc.partition_id_tensor.name if nc.partition_id_tensor else None
    in_names, out_names, out_avals = [], [], []
    for alloc in nc.m.functions[0].allocations:
        if not isinstance(alloc, mybir.MemoryLocationSet):
            continue
        name = alloc.memorylocations[0].name
        if alloc.kind == "ExternalInput":
            if name != partition_name:
                in_names.append(name)
        elif alloc.kind == "ExternalOutput":
            out_names.append(name)
            out_avals.append(jax.core.ShapedArray(
                tuple(alloc.tensor_shape), mybir.dt.np(alloc.dtype)))
    n_params, n_outs = len(in_names), len(out_avals)
    all_in_names = list(in_names) + list(out_names)
    if partition_name is not None:
        all_in_names.append(partition_name)

    def _body(*args):
        operands = list(args)
        if partition_name is not None:
            operands.append(partition_id_tensor())
        outs = _bass_exec_p.bind(
            *operands,
            out_avals=tuple(out_avals),
            in_names=tuple(all_in_names),
            out_names=tuple(out_names),
            lowering_input_output_aliases=(),
            sim_require_finite=True,
            sim_require_nnan=True,
            nc=nc,
        )
        return tuple(outs)

    devices = jax.devices()[:8]
    mesh = Mesh(np.asarray(devices), ("core",))
    sharded = jax.jit(
        shard_map(_body, mesh=mesh,
                  in_specs=(PartitionSpec("core"),) * (n_params + n_outs),
                  out_specs=(PartitionSpec("core"),) * n_outs,
                  check_rep=False),
        donate_argnums=tuple(range(n_params, n_params + n_outs)),
        keep_unused=True,
    )
    sharding = jax.sharding.NamedSharding(mesh, PartitionSpec("core"))
    runner = dict(nc=nc, sharded=sharded, sharding=sharding, jax=jax,
                  in_names=in_names, out_names=out_names, out_avals=out_avals)
    _CACHE[modes] = runner
    return runner


def _vec_tile(v, m_tiles):
    """[C] -> [128, m_tiles] column-per-m-tile layout."""
    return np.ascontiguousarray(np.asarray(v).reshape(m_tiles, 128).T
                                .astype(np.float32))


def prepare(w1, w2, w3, g1, b1, m1, v1, g2, b2, m2, v2, g3, b3, m3, v3):
    """Host prep: returns (modes, shared_input_dict_without_x)."""
    s1 = g1 / np.sqrt(v1 + EPS)
    t1 = b1 - m1 * s1
    s2 = g2 / np.sqrt(v2 + EPS)
    t2 = b2 - m2 * s2
    s3 = g3 / np.sqrt(v3 + EPS)
    t3 = b3 - m3 * s3

    def mode_of(s, t):
        if np.all(s > 0):
            return FAST_T0 if not np.any(t) else FAST_T
        return SLOW

    modes = (mode_of(s1, t1), mode_of(s2, t2), mode_of(s3, t3))

    def sc_bi(mode, s, m_tiles):
        if mode == SLOW:
            return _vec_tile(s, m_tiles), np.ones((128, m_tiles), np.float32)
        r = np.sqrt(s)
        return _vec_tile(r, m_tiles), _vec_tile(r, m_tiles)

    sc1, bi1 = sc_bi(modes[0], s1, M1)
    sc2, bi2 = sc_bi(modes[1], s2, M1)
    sc3, bi3 = sc_bi(modes[2], s3, M3)
    scb = np.concatenate([sc1, bi1, sc2, bi2, sc3, bi3], axis=1)
    shb = np.concatenate([_vec_tile(t1, M1), _vec_tile(t2, M1),
                          _vec_tile(t3, M3)], axis=1)

    w1t = np.ascontiguousarray(w1[:, :, 0, 0].T.astype(np.float32))      # [1024,256]
    # w2: [o, i, kh, kw] -> [k, tap, i_local, o]
    w2t = np.ascontiguousarray(
        w2.transpose(1, 2, 3, 0)                  # [i, kh, kw, o]
          .reshape(K2, 128, 9, C_MID)             # [k, i_local, tap, o]
          .transpose(0, 2, 1, 3)                  # [k, tap, i_local, o]
          .astype(np.float32))
    w3t = np.ascontiguousarray(w3[:, :, 0, 0].T.astype(np.float32))      # [256,1024]

    shared = dict(w1t=w1t, w2t=w2t, w3t=w3t, scb=scb, shb=shb)
    return modes, shared


def kernel(**inputs):
    inputs = {k: np.asarray(v) for k, v in inputs.items()}
    x = inputs.pop("x").astype(np.float32)
    modes, shared = prepare(**inputs)
    r = _get_runner(modes)
    jax = r["jax"]

    n_cores = 8
    dev_in = []
    for name in r["in_names"]:
        if name == "x":
            cat = x  # [128, ...] == 8 cores x 16
        else:
            a = shared[name]
            cat = np.concatenate([a] * n_cores, axis=0)
        dev_in.append(jax.device_put(cat, r["sharding"]))
    zero_outs = [
        jax.device_put(np.zeros((n_cores * av.shape[0], *av.shape[1:]), av.dtype),
                       r["sharding"])
        for av in r["out_avals"]
    ]
    outs = r["sharded"](*dev_in, *zero_outs)
    jax.block_until_ready(outs)
    out = np.asarray(outs[r["out_names"].index("out")])
    return out.reshape(128, C_IN, HW, HW)
